# revision 50
# baseline (speedup 1.0000x reference)
"""Trainium2 Bass kernel for nn_Attention (B=4, S=1024, DIM=1024, H=16, Dh=64).

Sharding: 8 cores = 4 batches x 2 head-groups (8 heads / 512 inner channels
each).  Each core computes q/k/v projections for its head shard, RoPE,
attention, and a partial output projection (its rows of Wo); the host sums
the two head-group partials per batch (the tensor-parallel all-reduce done
on host) and concatenates batches.

Device dataflow (per core), matmul operands in fp16 (fp32 PSUM accumulate):
  x^T staged in SBUF ->
  Q^T,K^T = W^T @ x^T      (bias added on the PSUM->SBUF pass)
  RoPE on the first 64 flat channels only (reference rotates rot_dim=64 of
  the flat inner dim): qr = (q+b)*cos + P_rot@((q+b)*sin), P_rot on PE.
  scores^T[k,q] = K_h @ Q_h^T   (K=64; the two heads of a row-tile issue
                                 back-to-back on row groups 0/64 -> concurrent)
  P^T = exp(scores^T/8 + maskbias[k])  (ACT, one op per head over q=1024;
                                        key mask folded into the exp bias)
  attn^T[c,q] (+rowsum via a ones-column in V_aug) = V_aug^T @ P^T
  rowsums gathered -> one batched reciprocal -> DMA partition-broadcast ->
  normalize -> out[q,:] = attn^T.T @ Wo_shard + bo/2 (K=1 matmul), masked
  rows zeroed on the PSUM->SBUF copy.
"""

import numpy as np

B, S, DIM, HEADS, HEAD_DIM = 4, 1024, 1024, 16, 64
INNER = HEADS * HEAD_DIM
HG = 2                      # head groups (tensor-parallel shards)
DSH = INNER // HG           # 512 inner channels per core
HSH = HEADS // HG           # 8 heads per core
NCORES = B * HG
KT = DIM // 128             # 8 contraction tiles
MT = DSH // 128             # 4 output row tiles for Q^T/K^T
ST = S // 128               # 8 seq tiles
MASK_NEG = -80.0

_CACHE = {}


def _build():
    import concourse.tile as tile
    from concourse import bacc, mybir

    f32 = mybir.dt.float32
    f16 = mybir.dt.float16
    AF = mybir.ActivationFunctionType
    OP = mybir.AluOpType

    nc = bacc.Bacc("TRN2", target_bir_lowering=False, debug=False)

    xT_d = nc.dram_tensor("xT", [128, KT, S], f16, kind="ExternalInput")
    wq_d = nc.dram_tensor("wq", [128, KT, MT, 128], f16, kind="ExternalInput")
    wk_d = nc.dram_tensor("wk", [128, KT, MT, 128], f16, kind="ExternalInput")
    wv_d = nc.dram_tensor("wv", [128, KT, DSH], f16, kind="ExternalInput")
    wo_d = nc.dram_tensor("wo", [128, MT, DIM], f16, kind="ExternalInput")
    bq_d = nc.dram_tensor("bq", [128, MT], f32, kind="ExternalInput")
    bk_d = nc.dram_tensor("bk", [128, MT], f32, kind="ExternalInput")
    bv_d = nc.dram_tensor("bv", [1, DSH], f16, kind="ExternalInput")
    bo_d = nc.dram_tensor("bo", [1, DIM], f16, kind="ExternalInput")
    cos_d = nc.dram_tensor("cos2", [128, S], f32, kind="ExternalInput")
    sin_d = nc.dram_tensor("sin2", [128, S], f32, kind="ExternalInput")
    prt_d = nc.dram_tensor("prt", [128, 128], f16, kind="ExternalInput")
    maskb_d = nc.dram_tensor("maskb", [128, ST], f32, kind="ExternalInput")
    mask01_d = nc.dram_tensor("mask01", [128, ST], f32, kind="ExternalInput")
    out_d = nc.dram_tensor("out", [S, DIM], f32, kind="ExternalOutput")

    with tile.TileContext(nc) as tc, \
         tc.tile_pool(name="persist", bufs=1) as persist:
        with tc.tile_pool(name="w1", bufs=1) as w1:
            # phase-1-only constants
            xT = w1.tile([128, KT, S], f16)
            wq = w1.tile([128, KT, MT, 128], f16)
            wk = w1.tile([128, KT, MT, 128], f16)
            wv = w1.tile([128, KT, DSH], f16)
            bq = w1.tile([128, MT], f32)
            bk = w1.tile([128, MT], f32)
            bv = w1.tile([1, DSH], f16)
            cos2 = w1.tile([128, S], f32)
            sin2 = w1.tile([128, S], f32)
            prt = w1.tile([128, 128], f16)
            # big per-tensor DMAs (each spreads over all 16 SDMA engines),
            # issued from different engine queues so they don't serialize
            # on one HWDGE FIFO.
            nc.sync.dma_start(out=xT[:, 0:4], in_=xT_d.ap()[:, 0:4])
            nc.sync.dma_start(out=xT[:, 4:8], in_=xT_d.ap()[:, 4:8])
            nc.scalar.dma_start(out=wk[:], in_=wk_d.ap())
            nc.gpsimd.dma_start(out=wq[:], in_=wq_d.ap())
            nc.scalar.dma_start(out=wv[:], in_=wv_d.ap())
            for t, d in [(bq, bq_d), (bk, bk_d), (bv, bv_d),
                         (cos2, cos_d), (sin2, sin_d), (prt, prt_d)]:
                nc.gpsimd.dma_start(out=t[:], in_=d.ap())
            # persistent across phases
            wo = persist.tile([128, MT, DIM], f16)
            bo = persist.tile([1, DIM], f16)
            maskb = persist.tile([128, ST], f32)
            mask01 = persist.tile([128, ST], f32)
            ones = persist.tile([1, S], f16)
            nc.scalar.dma_start(out=wo[:], in_=wo_d.ap())
            for t, d in [(bo, bo_d), (maskb, maskb_d), (mask01, mask01_d)]:
                nc.gpsimd.dma_start(out=t[:], in_=d.ap())
            ones_f = w1.tile([128, S], f32)
            nc.vector.memset(ones_f[:], 1.0)
            nc.vector.tensor_copy(ones[:], ones_f[0:1, :])

            qT = persist.tile([128, MT, S], f16)
            kT = persist.tile([128, MT, S], f16)
            vv = persist.tile([128, ST, HSH, HEAD_DIM], f16)
            ones_col = persist.tile([128, 1], f16)
            nc.vector.tensor_copy(ones_col[:], ones_f[:, 0:1])
            ones4 = persist.tile([97, HEAD_DIM], f16)
            nc.vector.tensor_copy(ones4[:], ones_f[0:97, 0:HEAD_DIM])

            # ---- phases 1+2: projections zippered into attention ----
            # K0/Q0/V run as a prologue; each attention row-tile's inner
            # loop then carries the NEXT row-tile's 32 projection matmuls
            # (4 per key-chunk) so PE fills the gaps while ACT streams exps.
            attU = persist.tile([128, MT, S], f16)
            # rowsums at partitions 0/32/64/96 (col-group constraint)
            rssum = persist.tile([97, MT, 512], f32)
            recq = persist.tile([97, MT, 512], f16)
            recf = persist.tile([97, MT, 512], f32)
            recd = nc.dram_tensor("recd", [97, MT, 512], f16)

            with tc.tile_pool(name="p1ps", bufs=1, space="PSUM") as p1ps, \
                 tc.tile_pool(name="p1sb", bufs=3) as p1sb, \
                 tc.tile_pool(name="p2r", bufs=3) as p2r:

                def rope_apply(dst, b, c2, ps, pppool):
                    # row-tile 0 only: RoPE on the first 64 flat channels
                    # (rows 64-127 and the hg=1 core get identity via
                    # cos=1/sin=0 from the host).
                    sl = slice(c2 * 512, (c2 + 1) * 512)
                    sinp = p1sb.tile([128, 512], f16, tag="sinp", name="sinp")
                    nc.vector.scalar_tensor_tensor(
                        sinp[:], ps[:], b[:, 0:1],
                        sin2[:, sl], op0=OP.add, op1=OP.mult)
                    cosp = p1sb.tile([128, 512], f32, tag="cosp", name="cosp")
                    nc.vector.scalar_tensor_tensor(
                        cosp[:], ps[:], b[:, 0:1],
                        cos2[:, sl], op0=OP.add, op1=OP.mult)
                    pp = pppool.tile([128, 512], f32, tag="pp", name="pp")
                    nc.tensor.matmul(out=pp[:], lhsT=prt[:], rhs=sinp[:],
                                     start=True, stop=True)
                    nc.vector.tensor_tensor(
                        dst[:, 0, sl], cosp[:], pp[:], op=OP.add)

                def proj_v(st, pool):
                    ps = pool.tile([128, DSH], f32, tag="vps", name="ps")
                    nc.tensor.matmul(out=ps[:], lhsT=ones[0:1, 0:128],
                                     rhs=bv[:], start=True, stop=False)
                    for kt in range(KT):
                        nc.tensor.matmul(
                            out=ps[:],
                            lhsT=xT[:, kt, st * 128:(st + 1) * 128],
                            rhs=wv[:, kt, :],
                            start=False, stop=(kt == KT - 1))
                    # spill on DVE: ACT's FIFO must stay clear so the first
                    # attention exps aren't queued behind these copies
                    nc.vector.tensor_copy(
                        vv[:, st, :, :],
                        ps[:].rearrange("p (h d) -> p h d", h=HSH))

                def proj_gen(mt):
                    # generator: one projection matmul per next(); bias-add
                    # epilogue rides with each group's final matmul.
                    for dst, w, b in ((kT, wk, bk), (qT, wq, bq)):
                        for c2 in range(2):
                            sl = slice(c2 * 512, (c2 + 1) * 512)
                            ps = p1ps.tile([128, 512], f32, tag="ps",
                                           name="ps")
                            for kt in range(KT):
                                nc.tensor.matmul(
                                    out=ps[:], lhsT=w[:, kt, mt, :],
                                    rhs=xT[:, kt, sl],
                                    start=(kt == 0), stop=(kt == KT - 1))
                                if kt == KT - 1:
                                    nc.vector.tensor_scalar(
                                        dst[:, mt, sl], ps[:], b[:, mt:mt + 1],
                                        None, op0=OP.add)
                                yield
                    while True:
                        yield

                # prologue: row-tile 0 projections + all of V, with a
                # deep PSUM pool (banks are free until the attention pools
                # open); all 32 K0/Q0 matmuls run dense, then the RoPE
                # chains consume the held PSUM tiles while V streams.
                with tc.tile_pool(name="p1pp", bufs=2,
                                  space="PSUM") as p1pp, \
                     tc.tile_pool(name="vps", bufs=5,
                                  space="PSUM") as vps:
                    kq_ps = []
                    for dst, w, b in ((kT, wk, bk), (qT, wq, bq)):
                        for c2 in range(2):
                            sl = slice(c2 * 512, (c2 + 1) * 512)
                            ps = vps.tile([128, 512], f32, tag="vps",
                                          name="ps")
                            for kt in range(KT):
                                nc.tensor.matmul(
                                    out=ps[:], lhsT=w[:, kt, 0, :],
                                    rhs=xT[:, kt, sl],
                                    start=(kt == 0), stop=(kt == KT - 1))
                            kq_ps.append((dst, b, c2, ps))
                    proj_v(0, vps)
                    for dst, b, c2, ps in kq_ps:
                        rope_apply(dst, b, c2, ps, p1pp)
                    for st in range(1, ST):
                        proj_v(st, vps)

                def normalize(mt):
                    # DRAM bounce on the gpsimd DMA queue partition-
                    # broadcasts each head's 1/rowsum row; the scale runs on
                    # the otherwise-idle GpSimd engine to keep DVE clear.
                    nc.gpsimd.dma_start(out=recd.ap()[:, mt, :],
                                        in_=recq[:, mt, :])
                    for hh in range(2):
                        ph = hh * 64
                        rb = p2r.tile([128, 2, 512], f32, tag="rb", name="rb")
                        nc.gpsimd.dma_start(
                            out=rb[ph:ph + 64],
                            in_=recd.ap()[64 * hh:64 * hh + 33:32,
                                          mt, :].partition_broadcast(HEAD_DIM))
                        nc.gpsimd.tensor_tensor(
                            attU[ph:ph + 64, mt, :], attU[ph:ph + 64, mt, :],
                            rb[ph:ph + 64].rearrange("p a b -> p (a b)"),
                            op=OP.mult)

                with tc.tile_pool(name="p2sc", bufs=1, space="PSUM") as p2sc, \
                     tc.tile_pool(name="p2at", bufs=1, space="PSUM") as p2at, \
                     tc.tile_pool(name="p2sb", bufs=3) as p2sb:

                    def emit_scores(mt, kt, c2s=(0, 1)):
                        sch = {}
                        for c2 in c2s:
                            qsl = slice(c2 * 512, (c2 + 1) * 512)
                            for hh in range(2):  # adjacent pair -> concurrent
                                ph = hh * 64
                                sch[hh, c2] = p2sc.tile(
                                    [128, 512], f32,
                                    tag=f"sc{hh}{c2}", name=f"sc{hh}{c2}")
                                nc.tensor.matmul(
                                    out=sch[hh, c2][:],
                                    lhsT=kT[ph:ph + 64, mt,
                                            kt * 128:(kt + 1) * 128],
                                    rhs=qT[ph:ph + 64, mt, qsl],
                                    start=True, stop=True,
                                    tile_position=(ph, 0))
                        return sch

                    for mt in range(MT):
                        gen = proj_gen(mt + 1) if mt + 1 < MT else iter(
                            lambda: None, 0)  # infinite no-op iterator
                        at = {c2: p2at.tile([128, 512], f32, name=f"at{c2}",
                                            tag=f"at{c2}")
                              for c2 in range(2)}
                        rsps = p2at.tile([97, 512], f32, tag="rsps",
                                         name="rsps")
                        sch = emit_scores(mt, 0)
                        for kt in range(ST):
                            pt = {}
                            for c2 in range(2):
                                for hh in range(2):
                                    pt[hh, c2] = p2sb.tile(
                                        [128, 512], f16,
                                        tag=f"pt{hh}{c2}", name=f"pt{hh}{c2}")
                                    nc.scalar.activation(
                                        pt[hh, c2][:], sch[hh, c2][:], AF.Exp,
                                        bias=maskb[:, kt:kt + 1], scale=0.125)
                            first, last = (kt == 0), (kt == ST - 1)
                            nproj = (5, 5, 5, 5, 4, 4, 4, 0)[kt]
                            for _ in range(nproj - nproj // 2):
                                next(gen)
                            sch_n = {}
                            for c2 in range(2):
                                qsl = slice(c2 * 512, (c2 + 1) * 512)
                                if not last:
                                    for hh in range(2):  # paired scores
                                        ph = hh * 64
                                        sch_n[hh, c2] = p2sc.tile(
                                            [128, 512], f32,
                                            tag=f"sc{hh}{c2}",
                                            name=f"sc{hh}{c2}")
                                        nc.tensor.matmul(
                                            out=sch_n[hh, c2][:],
                                            lhsT=kT[ph:ph + 64, mt,
                                                    (kt + 1) * 128:
                                                    (kt + 2) * 128],
                                            rhs=qT[ph:ph + 64, mt, qsl],
                                            start=True, stop=True,
                                            tile_position=(ph, 0))
                                for hh in range(2):  # PV pair: cols 0/64
                                    nc.tensor.matmul(
                                        out=at[c2][hh * 64:hh * 64 + 64, :],
                                        lhsT=vv[:, kt, mt * 2 + hh, :],
                                        rhs=pt[hh, c2][:],
                                        start=first, stop=last,
                                        tile_position=(0, hh * 64))
                                for hh in range(2):  # rowsum pair: cols r
                                    r = 32 * (hh * 2 + c2)
                                    nc.tensor.matmul(
                                        out=rsps[r:r + 1, :],
                                        lhsT=ones_col[:],
                                        rhs=pt[hh, c2][:],
                                        start=first, stop=last,
                                        tile_position=(0, r))
                                if c2 == 0:
                                    for _ in range(nproj // 2):
                                        next(gen)
                            sch = sch_n
                            if kt == 2 and mt > 0:
                                normalize(mt - 1)
                        # epilogue: rowsums + reciprocal first (they gate
                        # the normalize chain), attn spill after
                        for hh in range(2):
                            for c2 in range(2):
                                r = 32 * (hh * 2 + c2)
                                nc.vector.tensor_copy(
                                    rssum[r:r + 1, mt, :], rsps[r:r + 1, :])
                        # junk partitions between the four used rows are
                        # reciprocal'd too and ignored; inputs are well away
                        # from the approx-recip edge cases
                        nc.vector.reciprocal_approx_fast(
                            recf[:, mt, :], rssum[:, mt, :])
                        nc.vector.tensor_copy(recq[:, mt, :], recf[:, mt, :])
                        for c2 in range(2):
                            qsl = slice(c2 * 512, (c2 + 1) * 512)
                            if mt == MT - 1 and c2 == 0:
                                # ACT idles once the last exp retires; run
                                # the two spills on ACT and DVE in parallel
                                nc.scalar.activation(attU[:, mt, qsl],
                                                     at[c2][:], AF.Copy)
                            else:
                                nc.vector.tensor_copy(attU[:, mt, qsl],
                                                      at[c2][:])
                    # last row-tile: normalize via PE broadcast (the DRAM
                    # bounce's DMA latency would sit fully exposed here);
                    # c2-outer so the first output q-tiles unblock early
                    for c2 in range(2):
                        for hh in range(2):
                            ph = hh * 64
                            r = 32 * (hh * 2 + c2)
                            qsl = slice(c2 * 512, (c2 + 1) * 512)
                            rbps = p1ps.tile([HEAD_DIM, 512], f32, tag="ps",
                                             name="rbps")
                            nc.tensor.matmul(
                                out=rbps[:], lhsT=ones4[r:r + 1, :],
                                rhs=recq[r:r + 1, MT - 1, :],
                                start=True, stop=True, tile_position=(r, 0))
                            nc.vector.tensor_tensor(
                                attU[ph:ph + 64, MT - 1, qsl],
                                attU[ph:ph + 64, MT - 1, qsl],
                                rbps[:], op=OP.mult)

        # ---- phase 3: output projection -------------------------------
        with tc.tile_pool(name="p3ps", bufs=8, space="PSUM") as p3ps, \
             tc.tile_pool(name="p3sb", bufs=4) as p3sb:
            pre = {}
            for qt in range(4):
                # pre-start the first 8 groups' bias matmuls: they depend
                # only on bo, so they execute during the mt3 normalize
                # chain and keep the PE clock warm
                for c2 in range(2):
                    nsl = slice(c2 * 512, (c2 + 1) * 512)
                    ps = p3ps.tile([128, 512], f32, tag="ps3", name="ps3")
                    nc.tensor.matmul(
                        out=ps[:], lhsT=ones[0:1, 0:128], rhs=bo[0:1, nsl],
                        start=True, stop=False)
                    pre[qt, c2] = ps
            for qt in range(ST):
                ob = p3sb.tile([128, DIM], f32, tag="ob")
                for c2 in range(DIM // 512):
                    nsl = slice(c2 * 512, (c2 + 1) * 512)
                    if (qt, c2) in pre:
                        ps = pre[qt, c2]
                    else:
                        ps = p3ps.tile([128, 512], f32, tag="ps3",
                                       name="ps3")
                        nc.tensor.matmul(
                            out=ps[:], lhsT=ones[0:1, 0:128],
                            rhs=bo[0:1, nsl], start=True, stop=False)
                    for mt in range(MT):
                        nc.tensor.matmul(
                            out=ps[:],
                            lhsT=attU[:, mt, qt * 128:(qt + 1) * 128],
                            rhs=wo[:, mt, nsl],
                            start=False, stop=(mt == MT - 1))
                    # masked-row zeroing fused into the PSUM->SBUF move, on
                    # ACT (idle in this phase) to keep DVE off the path
                    nc.scalar.activation(
                        ob[:, nsl], ps[:], AF.Copy,
                        scale=mask01[:, qt:qt + 1])
                    eng = nc.sync if c2 == 0 else nc.scalar
                    eng.dma_start(
                        out=out_d.ap()[qt * 128:(qt + 1) * 128, nsl],
                        in_=ob[:, nsl])

    nc.compile()
    return nc


def _get_nc():
    if "nc" not in _CACHE:
        _CACHE["nc"] = _build()
    return _CACHE["nc"]


def _prep_inputs(x, mask, freqs, Wq, bq, Wk, bk, Wv, bv, Wo, bo):
    f = np.asarray(freqs, np.float32)[0]              # [S, HEAD_DIM]
    # reference rotates only the first rot_dim=64 channels of the FLAT
    # inner dim -> rows 0-63 of row-tile 0 on the hg=0 core; everything
    # else is identity (cos=1, sin=0).
    cos2 = np.ones((128, S), np.float32)
    sin2 = np.zeros((128, S), np.float32)
    cos2[0:HEAD_DIM] = np.cos(f.T)
    sin2[0:HEAD_DIM] = np.sin(f.T)
    ident = np.ones((128, S), np.float32)
    identz = np.zeros((128, S), np.float32)

    prt = np.zeros((128, 128), np.float16)            # P_rot^T
    i = np.arange(0, 128, 2)
    prt[i + 1, i] = -1.0                              # P_rot[2i, 2i+1] = -1
    prt[i, i + 1] = 1.0                               # P_rot[2i+1, 2i] = +1

    def lhsT_w(w):                                    # [DIM, DSH] -> lhsT tiles
        return np.ascontiguousarray(
            w.reshape(KT, 128, MT, 128).transpose(1, 0, 2, 3)).astype(np.float16)

    def col(b):                                       # [DSH] -> [128, MT]
        return np.ascontiguousarray(b.reshape(MT, 128).T.astype(np.float32))

    in_maps = []
    for b in range(B):
        xT = np.ascontiguousarray(
            np.asarray(x[b], np.float32).T.reshape(KT, 128, S)
            .transpose(1, 0, 2)).astype(np.float16)
        m = np.asarray(mask[b])
        maskb = np.ascontiguousarray(
            np.where(m, 0.0, MASK_NEG).astype(np.float32).reshape(ST, 128).T)
        mask01 = np.ascontiguousarray(
            m.astype(np.float32).reshape(ST, 128).T)
        for hg in range(HG):
            dsl = slice(hg * DSH, (hg + 1) * DSH)
            in_maps.append({
                "xT": xT,
                "wq": lhsT_w(np.asarray(Wq, np.float32)[:, dsl]),
                "wk": lhsT_w(np.asarray(Wk, np.float32)[:, dsl]),
                "wv": np.ascontiguousarray(
                    np.asarray(Wv, np.float32)[:, dsl]
                    .reshape(KT, 128, DSH).transpose(1, 0, 2)).astype(np.float16),
                "wo": np.ascontiguousarray(
                    np.asarray(Wo, np.float32)[dsl, :]
                    .reshape(MT, 128, DIM).transpose(1, 0, 2)).astype(np.float16),
                "bq": col(np.asarray(bq, np.float32)[dsl]),
                "bk": col(np.asarray(bk, np.float32)[dsl]),
                "bv": np.asarray(bv, np.float32)[None, dsl]
                    .astype(np.float16).copy(),
                "bo": (np.asarray(bo, np.float32) * 0.5)[None, :]
                    .astype(np.float16).copy(),
                "cos2": cos2 if hg == 0 else ident,
                "sin2": sin2 if hg == 0 else identz,
                "prt": prt,
                "maskb": maskb, "mask01": mask01,
            })
    return in_maps


def run(trace=False, **inputs):
    from concourse import bass_utils
    if trace:
        _install_ntff_hook()
    nc = _get_nc()
    in_maps = _prep_inputs(**inputs)
    res = bass_utils.run_bass_kernel_spmd(
        nc, in_maps, core_ids=list(range(NCORES)), trace=trace)
    out = np.empty((B, S, DIM), np.float32)
    for b in range(B):
        out[b] = res.results[2 * b]["out"] + res.results[2 * b + 1]["out"]
    return out, res


def kernel(**inputs):
    out, _ = run(trace=False, **inputs)
    return out


def _install_ntff_hook():
    """Register the axon NTFF profiling hook missing from the antenv stub."""
    import sys, types
    try:
        import antenv.axon_hooks  # noqa: F401
        return
    except ImportError:
        pass
    from trn_agent_boot.trn_boot import _ntff_profile_via_ctypes
    hook = _ntff_profile_via_ctypes('/opt/axon/libaxon_pjrt.so')
    mod = types.ModuleType('antenv.axon_hooks')
    mod.get_axon_ntff_profile_hook = lambda: hook
    mod.set_axon_ntff_profile_hook = lambda h: None
    sys.modules['antenv.axon_hooks'] = mod


# revision 52
# speedup vs baseline: 1.0432x; 1.0432x over previous
"""Trainium2 Bass kernel for nn_Attention (B=4, S=1024, DIM=1024, H=16, Dh=64).

Sharding: 8 cores = 4 batches x 2 head-groups (8 heads / 512 inner channels
each).  Each core computes q/k/v projections for its head shard, RoPE,
attention, and a partial output projection (its rows of Wo); the host sums
the two head-group partials per batch (the tensor-parallel all-reduce done
on host) and concatenates batches.

Device dataflow (per core), matmul operands in fp16 (fp32 PSUM accumulate):
  x^T staged in SBUF ->
  Q^T,K^T = W^T @ x^T      (bias added on the PSUM->SBUF pass)
  RoPE on the first 64 flat channels only (reference rotates rot_dim=64 of
  the flat inner dim): qr = (q+b)*cos + P_rot@((q+b)*sin), P_rot on PE.
  scores^T[k,q] = K_h @ Q_h^T   (K=64; the two heads of a row-tile issue
                                 back-to-back on row groups 0/64 -> concurrent)
  P^T = exp(scores^T/8 + maskbias[k])  (ACT, one op per head over q=1024;
                                        key mask folded into the exp bias)
  attn^T[c,q] (+rowsum via a ones-column in V_aug) = V_aug^T @ P^T
  rowsums gathered -> one batched reciprocal -> DMA partition-broadcast ->
  normalize -> out[q,:] = attn^T.T @ Wo_shard + bo/2 (K=1 matmul), masked
  rows zeroed on the PSUM->SBUF copy.
"""

import numpy as np

B, S, DIM, HEADS, HEAD_DIM = 4, 1024, 1024, 16, 64
INNER = HEADS * HEAD_DIM
HG = 2                      # head groups (tensor-parallel shards)
DSH = INNER // HG           # 512 inner channels per core
HSH = HEADS // HG           # 8 heads per core
NCORES = B * HG
KT = DIM // 128             # 8 contraction tiles
MT = DSH // 128             # 4 output row tiles for Q^T/K^T
ST = S // 128               # 8 seq tiles
MASK_NEG = -80.0

_CACHE = {}


def _build():
    import concourse.tile as tile
    from concourse import bacc, mybir

    f32 = mybir.dt.float32
    f16 = mybir.dt.float16
    AF = mybir.ActivationFunctionType
    OP = mybir.AluOpType

    nc = bacc.Bacc("TRN2", target_bir_lowering=False, debug=False)

    xT_d = nc.dram_tensor("xT", [128, KT, S], f16, kind="ExternalInput")
    wq_d = nc.dram_tensor("wq", [128, KT, MT, 128], f16, kind="ExternalInput")
    wk_d = nc.dram_tensor("wk", [128, KT, MT, 128], f16, kind="ExternalInput")
    wv_d = nc.dram_tensor("wv", [128, KT, DSH], f16, kind="ExternalInput")
    wo_d = nc.dram_tensor("wo", [128, MT, DIM], f16, kind="ExternalInput")
    bq_d = nc.dram_tensor("bq", [128, MT], f32, kind="ExternalInput")
    bk_d = nc.dram_tensor("bk", [128, MT], f32, kind="ExternalInput")
    bv_d = nc.dram_tensor("bv", [1, DSH], f16, kind="ExternalInput")
    bo_d = nc.dram_tensor("bo", [1, DIM], f16, kind="ExternalInput")
    cos_d = nc.dram_tensor("cos2", [128, S], f32, kind="ExternalInput")
    sin_d = nc.dram_tensor("sin2", [128, S], f32, kind="ExternalInput")
    prt_d = nc.dram_tensor("prt", [128, 128], f16, kind="ExternalInput")
    maskb_d = nc.dram_tensor("maskb", [128, ST], f32, kind="ExternalInput")
    mask01_d = nc.dram_tensor("mask01", [128, ST], f32, kind="ExternalInput")
    out_d = nc.dram_tensor("out", [S, DIM], f32, kind="ExternalOutput")

    with tile.TileContext(nc) as tc, \
         tc.tile_pool(name="persist", bufs=1) as persist:
        with tc.tile_pool(name="w1", bufs=1) as w1:
            # phase-1-only constants
            xT = w1.tile([128, KT, S], f16)
            wq = w1.tile([128, KT, MT, 128], f16)
            wk = w1.tile([128, KT, MT, 128], f16)
            wv = w1.tile([128, KT, DSH], f16)
            bq = w1.tile([128, MT], f32)
            bk = w1.tile([128, MT], f32)
            bv = w1.tile([1, DSH], f16)
            cos2 = w1.tile([128, S], f32)
            sin2 = w1.tile([128, S], f32)
            prt = w1.tile([128, 128], f16)
            # big per-tensor DMAs (each spreads over all 16 SDMA engines),
            # issued from different engine queues so they don't serialize
            # on one HWDGE FIFO.
            nc.sync.dma_start(out=xT[:, 0:4], in_=xT_d.ap()[:, 0:4])
            nc.sync.dma_start(out=xT[:, 4:8], in_=xT_d.ap()[:, 4:8])
            nc.scalar.dma_start(out=wk[:], in_=wk_d.ap())
            nc.gpsimd.dma_start(out=wq[:], in_=wq_d.ap())
            nc.scalar.dma_start(out=wv[:], in_=wv_d.ap())
            for t, d in [(bq, bq_d), (bk, bk_d), (bv, bv_d),
                         (cos2, cos_d), (sin2, sin_d), (prt, prt_d)]:
                nc.gpsimd.dma_start(out=t[:], in_=d.ap())
            # persistent across phases
            wo = persist.tile([128, MT, DIM], f16)
            bo = persist.tile([1, DIM], f16)
            maskb = persist.tile([128, ST], f32)
            mask01 = persist.tile([128, ST], f32)
            ones = persist.tile([1, S], f16)
            nc.scalar.dma_start(out=wo[:], in_=wo_d.ap())
            for t, d in [(bo, bo_d), (maskb, maskb_d), (mask01, mask01_d)]:
                nc.gpsimd.dma_start(out=t[:], in_=d.ap())
            ones_f = w1.tile([128, S], f32)
            nc.vector.memset(ones_f[:], 1.0)
            nc.vector.tensor_copy(ones[:], ones_f[0:1, :])

            qT = persist.tile([128, MT, S], f16)
            kT = persist.tile([128, MT, S], f16)
            vv = persist.tile([128, ST, HSH, HEAD_DIM], f16)
            ones_col = persist.tile([128, 1], f16)
            nc.vector.tensor_copy(ones_col[:], ones_f[:, 0:1])
            ones4 = persist.tile([97, HEAD_DIM], f16)
            nc.vector.tensor_copy(ones4[:], ones_f[0:97, 0:HEAD_DIM])

            # ---- phases 1+2: projections zippered into attention ----
            # K0/Q0/V run as a prologue; each attention row-tile's inner
            # loop then carries the NEXT row-tile's 32 projection matmuls
            # (4 per key-chunk) so PE fills the gaps while ACT streams exps.
            attU = persist.tile([128, MT, S], f16)
            # rowsums at partitions 0/32/64/96 (col-group constraint)
            rssum = persist.tile([97, MT, 512], f32)
            recq = persist.tile([97, MT, 512], f16)
            recf = persist.tile([97, MT, 512], f32)
            recd = nc.dram_tensor("recd", [97, MT, 512], f16)

            with tc.tile_pool(name="p1ps", bufs=1, space="PSUM") as p1ps, \
                 tc.tile_pool(name="p1sb", bufs=3) as p1sb, \
                 tc.tile_pool(name="p2r", bufs=2) as p2r:

                def rope_apply(dst, b, c2, ps, pppool):
                    # row-tile 0 only: RoPE on the first 64 flat channels
                    # (rows 64-127 and the hg=1 core get identity via
                    # cos=1/sin=0 from the host).
                    sl = slice(c2 * 512, (c2 + 1) * 512)
                    sinp = p1sb.tile([128, 512], f16, tag="sinp", name="sinp")
                    nc.vector.scalar_tensor_tensor(
                        sinp[:], ps[:], b[:, 0:1],
                        sin2[:, sl], op0=OP.add, op1=OP.mult)
                    cosp = p1sb.tile([128, 512], f32, tag="cosp", name="cosp")
                    nc.vector.scalar_tensor_tensor(
                        cosp[:], ps[:], b[:, 0:1],
                        cos2[:, sl], op0=OP.add, op1=OP.mult)
                    pp = pppool.tile([128, 512], f32, tag="pp", name="pp")
                    nc.tensor.matmul(out=pp[:], lhsT=prt[:], rhs=sinp[:],
                                     start=True, stop=True)
                    nc.vector.tensor_tensor(
                        dst[:, 0, sl], cosp[:], pp[:], op=OP.add)

                def proj_v(st, pool):
                    ps = pool.tile([128, DSH], f32, tag="vps", name="ps")
                    nc.tensor.matmul(out=ps[:], lhsT=ones[0:1, 0:128],
                                     rhs=bv[:], start=True, stop=False)
                    for kt in range(KT):
                        nc.tensor.matmul(
                            out=ps[:],
                            lhsT=xT[:, kt, st * 128:(st + 1) * 128],
                            rhs=wv[:, kt, :],
                            start=False, stop=(kt == KT - 1))
                    # spill on DVE: ACT's FIFO must stay clear so the first
                    # attention exps aren't queued behind these copies
                    nc.vector.tensor_copy(
                        vv[:, st, :, :],
                        ps[:].rearrange("p (h d) -> p h d", h=HSH))

                def proj_gen(mt):
                    # generator: one projection matmul per next(); bias-add
                    # epilogue rides with each group's final matmul.
                    for dst, w, b in ((kT, wk, bk), (qT, wq, bq)):
                        for c2 in range(2):
                            sl = slice(c2 * 512, (c2 + 1) * 512)
                            ps = p1ps.tile([128, 512], f32, tag="ps",
                                           name="ps")
                            for kt in range(KT):
                                nc.tensor.matmul(
                                    out=ps[:], lhsT=w[:, kt, mt, :],
                                    rhs=xT[:, kt, sl],
                                    start=(kt == 0), stop=(kt == KT - 1))
                                if kt == KT - 1:
                                    nc.vector.tensor_scalar(
                                        dst[:, mt, sl], ps[:], b[:, mt:mt + 1],
                                        None, op0=OP.add)
                                yield
                    while True:
                        yield

                # prologue: row-tile 0 projections + all of V, with a
                # deep PSUM pool (banks are free until the attention pools
                # open); all 32 K0/Q0 matmuls run dense, then the RoPE
                # chains consume the held PSUM tiles while V streams.
                with tc.tile_pool(name="p1pp", bufs=2,
                                  space="PSUM") as p1pp, \
                     tc.tile_pool(name="vps", bufs=5,
                                  space="PSUM") as vps:
                    kq_ps = []
                    for dst, w, b in ((kT, wk, bk), (qT, wq, bq)):
                        for c2 in range(2):
                            sl = slice(c2 * 512, (c2 + 1) * 512)
                            ps = vps.tile([128, 512], f32, tag="vps",
                                          name="ps")
                            for kt in range(KT):
                                nc.tensor.matmul(
                                    out=ps[:], lhsT=w[:, kt, 0, :],
                                    rhs=xT[:, kt, sl],
                                    start=(kt == 0), stop=(kt == KT - 1))
                            kq_ps.append((dst, b, c2, ps))
                    proj_v(0, vps)
                    for dst, b, c2, ps in kq_ps:
                        rope_apply(dst, b, c2, ps, p1pp)
                    for st in range(1, ST):
                        proj_v(st, vps)

                def normalize(mt):
                    # DRAM bounce on the gpsimd DMA queue partition-
                    # broadcasts each head's 1/rowsum row; the scale runs on
                    # the otherwise-idle GpSimd engine to keep DVE clear.
                    nc.gpsimd.dma_start(out=recd.ap()[:, mt, :],
                                        in_=recq[:, mt, :])
                    for hh in range(2):
                        ph = hh * 64
                        rb = p2r.tile([128, 2, 512], f32, tag="rb", name="rb")
                        nc.gpsimd.dma_start(
                            out=rb[ph:ph + 64],
                            in_=recd.ap()[64 * hh:64 * hh + 33:32,
                                          mt, :].partition_broadcast(HEAD_DIM))
                        nc.gpsimd.tensor_tensor(
                            attU[ph:ph + 64, mt, :], attU[ph:ph + 64, mt, :],
                            rb[ph:ph + 64].rearrange("p a b -> p (a b)"),
                            op=OP.mult)

                with tc.tile_pool(name="p2sc", bufs=1, space="PSUM") as p2sc, \
                     tc.tile_pool(name="p2at", bufs=1, space="PSUM") as p2at, \
                     tc.tile_pool(name="p2sb", bufs=2) as p2sb:

                    def emit_scores(mt, kt):
                        # one [128,1024] tile per q-chunk: h0 -> bank-half
                        # 0:512, h1 -> 512:1024 (distinct banks, adjacent
                        # row-group pair -> concurrent); both heads share
                        # the per-key exp bias so ONE exp covers the tile.
                        sch = {}
                        for c2 in range(2):
                            qsl = slice(c2 * 512, (c2 + 1) * 512)
                            sch[c2] = p2sc.tile([128, S], f32,
                                                tag=f"scc{c2}",
                                                name=f"scc{c2}")
                            for hh in range(2):
                                ph = hh * 64
                                nc.tensor.matmul(
                                    out=sch[c2][:, hh * 512:hh * 512 + 512],
                                    lhsT=kT[ph:ph + 64, mt,
                                            kt * 128:(kt + 1) * 128],
                                    rhs=qT[ph:ph + 64, mt, qsl],
                                    start=True, stop=True,
                                    tile_position=(ph, 0))
                        return sch

                    for mt in range(MT):
                        gen = proj_gen(mt + 1) if mt + 1 < MT else iter(
                            lambda: None, 0)  # infinite no-op iterator
                        at = {c2: p2at.tile([128, 512], f32, name=f"at{c2}",
                                            tag=f"at{c2}")
                              for c2 in range(2)}
                        rsps = p2at.tile([97, 512], f32, tag="rsps",
                                         name="rsps")
                        sch = emit_scores(mt, 0)
                        for kt in range(ST):
                            pt = {}
                            for c2 in range(2):
                                pt[c2] = p2sb.tile([128, S], f16,
                                                   tag=f"ptc{c2}",
                                                   name=f"ptc{c2}")
                                nc.scalar.activation(
                                    pt[c2][:], sch[c2][:], AF.Exp,
                                    bias=maskb[:, kt:kt + 1], scale=0.125)
                            first, last = (kt == 0), (kt == ST - 1)
                            nproj = (5, 5, 5, 5, 4, 4, 4, 0)[kt]
                            for _ in range(nproj - nproj // 2):
                                next(gen)
                            sch_n = {}
                            for c2 in range(2):
                                qsl = slice(c2 * 512, (c2 + 1) * 512)
                                if not last:
                                    sch_n[c2] = p2sc.tile(
                                        [128, S], f32,
                                        tag=f"scc{c2}", name=f"scc{c2}")
                                    for hh in range(2):  # paired scores
                                        ph = hh * 64
                                        nc.tensor.matmul(
                                            out=sch_n[c2][:, hh * 512:
                                                          hh * 512 + 512],
                                            lhsT=kT[ph:ph + 64, mt,
                                                    (kt + 1) * 128:
                                                    (kt + 2) * 128],
                                            rhs=qT[ph:ph + 64, mt, qsl],
                                            start=True, stop=True,
                                            tile_position=(ph, 0))
                                for hh in range(2):  # PV pair: cols 0/64
                                    nc.tensor.matmul(
                                        out=at[c2][hh * 64:hh * 64 + 64, :],
                                        lhsT=vv[:, kt, mt * 2 + hh, :],
                                        rhs=pt[c2][:, hh * 512:hh * 512 + 512],
                                        start=first, stop=last,
                                        tile_position=(0, hh * 64))
                                for hh in range(2):  # rowsum pair: cols r
                                    r = 32 * (hh * 2 + c2)
                                    nc.tensor.matmul(
                                        out=rsps[r:r + 1, :],
                                        lhsT=ones_col[:],
                                        rhs=pt[c2][:, hh * 512:hh * 512 + 512],
                                        start=first, stop=last,
                                        tile_position=(0, r))
                                if c2 == 0:
                                    for _ in range(nproj // 2):
                                        next(gen)
                            sch = sch_n
                            if kt == 2 and mt > 0:
                                normalize(mt - 1)
                        # epilogue: rowsums + reciprocal first (they gate
                        # the normalize chain), attn spill after
                        for hh in range(2):
                            for c2 in range(2):
                                r = 32 * (hh * 2 + c2)
                                nc.vector.tensor_copy(
                                    rssum[r:r + 1, mt, :], rsps[r:r + 1, :])
                        # junk partitions between the four used rows are
                        # reciprocal'd too and ignored; inputs are well away
                        # from the approx-recip edge cases
                        nc.vector.reciprocal_approx_fast(
                            recf[:, mt, :], rssum[:, mt, :])
                        nc.vector.tensor_copy(recq[:, mt, :], recf[:, mt, :])
                        for c2 in range(2):
                            qsl = slice(c2 * 512, (c2 + 1) * 512)
                            if mt == MT - 1 and c2 == 0:
                                # ACT idles once the last exp retires; run
                                # the two spills on ACT and DVE in parallel
                                nc.scalar.activation(attU[:, mt, qsl],
                                                     at[c2][:], AF.Copy)
                            else:
                                nc.vector.tensor_copy(attU[:, mt, qsl],
                                                      at[c2][:])
                    # last row-tile: normalize via PE broadcast (the DRAM
                    # bounce's DMA latency would sit fully exposed here);
                    # c2-outer so the first output q-tiles unblock early
                    for c2 in range(2):
                        for hh in range(2):
                            ph = hh * 64
                            r = 32 * (hh * 2 + c2)
                            qsl = slice(c2 * 512, (c2 + 1) * 512)
                            rbps = p1ps.tile([HEAD_DIM, 512], f32, tag="ps",
                                             name="rbps")
                            nc.tensor.matmul(
                                out=rbps[:], lhsT=ones4[r:r + 1, :],
                                rhs=recq[r:r + 1, MT - 1, :],
                                start=True, stop=True, tile_position=(r, 0))
                            nc.vector.tensor_tensor(
                                attU[ph:ph + 64, MT - 1, qsl],
                                attU[ph:ph + 64, MT - 1, qsl],
                                rbps[:], op=OP.mult)

        # ---- phase 3: output projection -------------------------------
        with tc.tile_pool(name="p3ps", bufs=8, space="PSUM") as p3ps, \
             tc.tile_pool(name="p3sb", bufs=3) as p3sb:
            pre = {}
            for qt in range(4):
                # pre-start the first 8 groups' bias matmuls: they depend
                # only on bo, so they execute during the mt3 normalize
                # chain and keep the PE clock warm
                for c2 in range(2):
                    nsl = slice(c2 * 512, (c2 + 1) * 512)
                    ps = p3ps.tile([128, 512], f32, tag="ps3", name="ps3")
                    nc.tensor.matmul(
                        out=ps[:], lhsT=ones[0:1, 0:128], rhs=bo[0:1, nsl],
                        start=True, stop=False)
                    pre[qt, c2] = ps
            for qt in range(ST):
                ob = p3sb.tile([128, DIM], f32, tag="ob")
                for c2 in range(DIM // 512):
                    nsl = slice(c2 * 512, (c2 + 1) * 512)
                    if (qt, c2) in pre:
                        ps = pre[qt, c2]
                    else:
                        ps = p3ps.tile([128, 512], f32, tag="ps3",
                                       name="ps3")
                        nc.tensor.matmul(
                            out=ps[:], lhsT=ones[0:1, 0:128],
                            rhs=bo[0:1, nsl], start=True, stop=False)
                    for mt in range(MT):
                        nc.tensor.matmul(
                            out=ps[:],
                            lhsT=attU[:, mt, qt * 128:(qt + 1) * 128],
                            rhs=wo[:, mt, nsl],
                            start=False, stop=(mt == MT - 1))
                    # masked-row zeroing fused into the PSUM->SBUF move, on
                    # ACT (idle in this phase) to keep DVE off the path
                    nc.scalar.activation(
                        ob[:, nsl], ps[:], AF.Copy,
                        scale=mask01[:, qt:qt + 1])
                    eng = nc.sync if c2 == 0 else nc.scalar
                    eng.dma_start(
                        out=out_d.ap()[qt * 128:(qt + 1) * 128, nsl],
                        in_=ob[:, nsl])

    nc.compile()
    return nc


def _get_nc():
    if "nc" not in _CACHE:
        _CACHE["nc"] = _build()
    return _CACHE["nc"]


def _prep_inputs(x, mask, freqs, Wq, bq, Wk, bk, Wv, bv, Wo, bo):
    f = np.asarray(freqs, np.float32)[0]              # [S, HEAD_DIM]
    # reference rotates only the first rot_dim=64 channels of the FLAT
    # inner dim -> rows 0-63 of row-tile 0 on the hg=0 core; everything
    # else is identity (cos=1, sin=0).
    cos2 = np.ones((128, S), np.float32)
    sin2 = np.zeros((128, S), np.float32)
    cos2[0:HEAD_DIM] = np.cos(f.T)
    sin2[0:HEAD_DIM] = np.sin(f.T)
    ident = np.ones((128, S), np.float32)
    identz = np.zeros((128, S), np.float32)

    prt = np.zeros((128, 128), np.float16)            # P_rot^T
    i = np.arange(0, 128, 2)
    prt[i + 1, i] = -1.0                              # P_rot[2i, 2i+1] = -1
    prt[i, i + 1] = 1.0                               # P_rot[2i+1, 2i] = +1

    def lhsT_w(w):                                    # [DIM, DSH] -> lhsT tiles
        return np.ascontiguousarray(
            w.reshape(KT, 128, MT, 128).transpose(1, 0, 2, 3)).astype(np.float16)

    def col(b):                                       # [DSH] -> [128, MT]
        return np.ascontiguousarray(b.reshape(MT, 128).T.astype(np.float32))

    in_maps = []
    for b in range(B):
        xT = np.ascontiguousarray(
            np.asarray(x[b], np.float32).T.reshape(KT, 128, S)
            .transpose(1, 0, 2)).astype(np.float16)
        m = np.asarray(mask[b])
        maskb = np.ascontiguousarray(
            np.where(m, 0.0, MASK_NEG).astype(np.float32).reshape(ST, 128).T)
        mask01 = np.ascontiguousarray(
            m.astype(np.float32).reshape(ST, 128).T)
        for hg in range(HG):
            dsl = slice(hg * DSH, (hg + 1) * DSH)
            in_maps.append({
                "xT": xT,
                "wq": lhsT_w(np.asarray(Wq, np.float32)[:, dsl]),
                "wk": lhsT_w(np.asarray(Wk, np.float32)[:, dsl]),
                "wv": np.ascontiguousarray(
                    np.asarray(Wv, np.float32)[:, dsl]
                    .reshape(KT, 128, DSH).transpose(1, 0, 2)).astype(np.float16),
                "wo": np.ascontiguousarray(
                    np.asarray(Wo, np.float32)[dsl, :]
                    .reshape(MT, 128, DIM).transpose(1, 0, 2)).astype(np.float16),
                "bq": col(np.asarray(bq, np.float32)[dsl]),
                "bk": col(np.asarray(bk, np.float32)[dsl]),
                "bv": np.asarray(bv, np.float32)[None, dsl]
                    .astype(np.float16).copy(),
                "bo": (np.asarray(bo, np.float32) * 0.5)[None, :]
                    .astype(np.float16).copy(),
                "cos2": cos2 if hg == 0 else ident,
                "sin2": sin2 if hg == 0 else identz,
                "prt": prt,
                "maskb": maskb, "mask01": mask01,
            })
    return in_maps


def run(trace=False, **inputs):
    from concourse import bass_utils
    if trace:
        _install_ntff_hook()
    nc = _get_nc()
    in_maps = _prep_inputs(**inputs)
    res = bass_utils.run_bass_kernel_spmd(
        nc, in_maps, core_ids=list(range(NCORES)), trace=trace)
    out = np.empty((B, S, DIM), np.float32)
    for b in range(B):
        out[b] = res.results[2 * b]["out"] + res.results[2 * b + 1]["out"]
    return out, res


def kernel(**inputs):
    out, _ = run(trace=False, **inputs)
    return out


def _install_ntff_hook():
    """Register the axon NTFF profiling hook missing from the antenv stub."""
    import sys, types
    try:
        import antenv.axon_hooks  # noqa: F401
        return
    except ImportError:
        pass
    from trn_agent_boot.trn_boot import _ntff_profile_via_ctypes
    hook = _ntff_profile_via_ctypes('/opt/axon/libaxon_pjrt.so')
    mod = types.ModuleType('antenv.axon_hooks')
    mod.get_axon_ntff_profile_hook = lambda: hook
    mod.set_axon_ntff_profile_hook = lambda h: None
    sys.modules['antenv.axon_hooks'] = mod
